# revision 1
# baseline (speedup 1.0000x reference)
"""GAT-style attention kernel for Trainium2, data-parallel over batch on 8 cores.

Math (see derivation in comments below): the reference computes
    e[i,j]  = lr_row[i] + lr_col[j]            (rank-1 score structure)
    atten   = softmax_j(where(mask>0, e, -1e9))
    out     = atten @ (x @ Wx.T + bx)
Because lr_row[i] is constant along the softmax axis j, it cancels:
    atten[i,j] = mask[i,j] * w[j] / sum_j mask[i,j] * w[j],
    w[j] = exp(lr_col[j] - max_j lr_col[j])
and since attention rows sum to 1, the bias bx passes through unchanged:
    out = (M @ (w * xv0)) / (M @ w) + bx,   xv0 = x @ Wx.T
So the whole kernel is one [N,N] x [N,129] matmul per batch, normalized
row-wise, with tiny setup.  Memory-bound on the int32 mask read (16MB/core).

Per core (batch b):
  - mask strips [128, N] are DMA-loaded with SWDGE int32->bf16 cast
  - xbar DMA-transpose produces maskT chunks [j_in, j_blk, i] in SBUF
  - PE accumulates psum[i, 132] over 16 j-chunks: lhsT=maskT chunk (bf16),
    rhs=U chunk [128, 132] where U[:, :128] = w*xv0, U[:, 128] = w
  - normalize by column 128, add bx, store f32
"""

import os
import sys

import numpy as np

for _p in ("/opt/trn_rl_repo",):
    if _p not in sys.path and os.path.isdir(_p):
        sys.path.append(_p)

import concourse.bacc as bacc
import concourse.bass as bass
import concourse.bass_isa as bass_isa
import concourse.tile as tile
from concourse import mybir
from concourse.bass_utils import run_bass_kernel_spmd

B, N, DIN, DOUT, DA = 8, 2048, 128, 128, 2
NEG_SLOPE = 0.2
P = 128
UC = 132  # U free width: 128 numerator cols + 1 denom col + 3 pad

F32 = mybir.dt.float32
BF16 = mybir.dt.bfloat16
I32 = mybir.dt.int32


def build(n=N, mask_bufs=6, use_3d_xbar=True, variant="hwdge_split", cast_cols_dve=2048,
          xpose_queues=("sync",), load_engine="alt"):
    """Build the single-core program (all 8 cores run it SPMD).

    variant:
      "swdge_cast":  SWDGE cast-DMA loads + xbar transposes on sync (v1; slow)
      "hwdge_split": plain int32 HWDGE loads, DVE+GpSimd cast, xbar transposes
                     split across sync+scalar queues
    """
    nt = n // P
    nc = bacc.Bacc(
        "TRN2",
        target_bir_lowering=False,
        debug=False,
        enable_asserts=False,
        num_devices=1,
    )
    x_d = nc.dram_tensor("x", [n, DIN], F32, kind="ExternalInput").ap()
    m_d = nc.dram_tensor("mask", [n, n], I32, kind="ExternalInput").ap()
    # wcomb = [Wx.T | Wc.T]  (precomputed on host; tiny params)
    wcomb_d = nc.dram_tensor("wcomb", [DIN, DOUT + DA], BF16, kind="ExternalInput").ap()
    a2_d = nc.dram_tensor("a2", [P, DA], F32, kind="ExternalInput").ap()
    bx_d = nc.dram_tensor("bx", [P, DOUT], F32, kind="ExternalInput").ap()
    ident_d = nc.dram_tensor("ident", [P, P], BF16, kind="ExternalInput").ap()
    out_d = nc.dram_tensor("out", [n, DOUT], F32, kind="ExternalOutput").ap()

    from contextlib import ExitStack

    with tile.TileContext(nc) as tc, ExitStack() as ctx:
        consts = ctx.enter_context(tc.tile_pool(name="consts", bufs=1))
        small = ctx.enter_context(tc.tile_pool(name="small", bufs=2))
        mpool = ctx.enter_context(tc.tile_pool(name="mpool", bufs=mask_bufs))
        cpool = ctx.enter_context(tc.tile_pool(name="cpool", bufs=max(2, mask_bufs - 1)))
        tpool = ctx.enter_context(tc.tile_pool(name="tpool", bufs=max(2, mask_bufs - 1)))
        opool = ctx.enter_context(tc.tile_pool(name="opool", bufs=3))
        ps_small = ctx.enter_context(tc.tile_pool(name="ps_small", bufs=2, space="PSUM"))
        ps_acc = ctx.enter_context(tc.tile_pool(name="ps_acc", bufs=4, space="PSUM"))

        # ---- constants (host pre-broadcast / pre-transposed) ----
        identB = consts.tile([P, P], BF16)
        nc.sync.dma_start(identB[:], ident_d)
        wcomb = consts.tile([DIN, DOUT + DA], BF16)
        nc.sync.dma_start(wcomb[:], wcomb_d)
        a2b = consts.tile([P, DA], F32)
        nc.sync.dma_start(a2b[:], a2_d)
        bxb = consts.tile([P, DOUT], F32)
        nc.sync.dma_start(bxb[:], bx_d)

        # ---- x -> xT (bf16) via PE transposes, packed 4/psum bank ----
        x_nat = consts.tile([P, nt, DIN], F32)
        nc.sync.dma_start(x_nat[:], x_d.rearrange("(t p) d -> p t d", p=P))
        xbf = consts.tile([P, nt * DIN], BF16)
        nc.vector.tensor_copy(xbf[:], x_nat[:].rearrange("p t d -> p (t d)"))
        xT = consts.tile([P, n], BF16)
        gs = 4 if nt % 4 == 0 else 1
        for g in range(nt // gs):
            psx = ps_small.tile([P, gs * P], BF16, tag="psx")
            for t4 in range(gs):
                t = g * gs + t4
                nc.tensor.transpose(
                    psx[:, t4 * P : (t4 + 1) * P],
                    xbf[:, t * DIN : (t + 1) * DIN],
                    identB[:],
                )
            nc.scalar.copy(xT[:, g * gs * P : (g + 1) * gs * P], psx[:])

        # ---- projections: pxv[j,130] = xT_chunk.T @ [WxT | WcT] ----
        xvcol = consts.tile([P, nt, DOUT + DA], F32)
        for t in range(nt):
            pxv = ps_small.tile([P, DOUT + DA], F32, tag="pxv")
            nc.tensor.matmul(
                pxv[:], xT[:, t * P : (t + 1) * P], wcomb[:], start=True, stop=True
            )
            nc.scalar.copy(xvcol[:, t], pxv[:])

        # ---- lr_col, global max, w = exp(lrc - max): whole-width ops ----
        colp = xvcol[:, :, DOUT : DOUT + DA]  # [P, nt, 2] strided view
        c02 = small.tile([P, nt, DA], F32)
        nc.vector.tensor_scalar_mul(c02[:], colp, NEG_SLOPE)
        clr = small.tile([P, nt, DA], F32)
        nc.vector.tensor_max(clr[:], colp, c02[:])
        lr0 = small.tile([P, nt], F32)
        nc.vector.tensor_scalar(
            lr0[:], clr[:, :, 0], a2b[:, 0:1], None, mybir.AluOpType.mult
        )
        lr1 = small.tile([P, nt], F32)
        nc.vector.tensor_scalar(
            lr1[:], clr[:, :, 1], a2b[:, 1:2], None, mybir.AluOpType.mult
        )
        lrc = small.tile([P, nt], F32)
        nc.vector.tensor_add(lrc[:], lr0[:], lr1[:])
        mx = small.tile([P, 1], F32)
        nc.vector.tensor_reduce(
            mx[:], lrc[:], axis=mybir.AxisListType.X, op=mybir.AluOpType.max
        )
        mxr = small.tile([P, 1], F32)
        nc.gpsimd.partition_all_reduce(
            mxr[:], mx[:], channels=P, reduce_op=bass_isa.ReduceOp.max
        )
        negmx = small.tile([P, 1], F32)
        nc.vector.tensor_scalar_mul(negmx[:], mxr[:], -1.0)
        w_all = consts.tile([P, nt], F32)
        nc.scalar.activation(
            w_all[:], lrc[:], mybir.ActivationFunctionType.Exp, bias=negmx[:]
        )

        # ---- U chunks [P, nt, UC] bf16: U[:,:,0:128]=w*xv, U[:,:,128]=w ----
        U = consts.tile([P, nt, UC], BF16)
        nc.vector.memset(U[:], 0)
        for t in range(nt):
            nc.scalar.activation(
                U[:, t, 0:DOUT],
                xvcol[:, t, 0:DOUT],
                mybir.ActivationFunctionType.Copy,
                scale=w_all[:, t : t + 1],
            )
        nc.vector.tensor_copy(U[:, :, DOUT], w_all[:])

        raw = consts.tile([P, nt, UC], F32)

        # ---- main loop over output row strips ----
        paccs = []
        for ti in range(nt):
            if variant == "swdge_cast":
                mbf = mpool.tile([P, n], BF16)
                nc.gpsimd.dma_start(mbf[:], m_d[ti * P : (ti + 1) * P, :])
                mT = tpool.tile([P, nt, P], BF16)
                if use_3d_xbar:
                    nc.sync.dma_start(mT[:], mbf[:], transpose=True)
                else:
                    for tj in range(nt):
                        nc.sync.dma_start(
                            mT[:, tj], mbf[:, tj * P : (tj + 1) * P], transpose=True
                        )
            else:
                mi32 = mpool.tile([P, n], I32)
                # sync (SP) queue is load-only: its waits never gate compute
                nc.sync.dma_start(mi32[:], m_d[ti * P : (ti + 1) * P, :])
                mbf = cpool.tile([P, n], BF16)
                cc = max(P, min(n, cast_cols_dve * n // N))
                nc.vector.tensor_copy(mbf[:, 0:cc], mi32[:, 0:cc])
                if cc < n:
                    nc.gpsimd.tensor_copy(mbf[:, cc:n], mi32[:, cc:n])
                mT = tpool.tile([P, nt, P], BF16)
                # scalar (ACT) queue is transpose-only during the main loop
                nc.scalar.dma_start(mT[:], mbf[:], transpose=True)
            pacc = ps_acc.tile([P, UC], F32)
            paccs.append(pacc)
            for tj in range(nt):
                nc.tensor.matmul(
                    pacc[:],
                    mT[:, tj],
                    U[:, tj],
                    start=(tj == 0),
                    stop=(tj == nt - 1),
                )
            # evacuate PSUM on DVE with a 2-strip skew: by the time the copy
            # appears in DVE's program, the MMs it waits on are long done
            if ti >= 3:
                nc.vector.tensor_copy(raw[:, ti - 3], paccs[ti - 3][:])
        for ti in range(max(0, nt - 3), nt):
            nc.vector.tensor_copy(raw[:, ti], paccs[ti][:])

        # ---- phase B: normalize + bias + store ----
        for ti in range(nt):
            rec = small.tile([P, 1], F32)
            nc.vector.reciprocal(rec[:], raw[:, ti, DOUT : DOUT + 1])
            o1 = opool.tile([P, DOUT], F32)
            nc.scalar.activation(
                o1[:], raw[:, ti, 0:DOUT], mybir.ActivationFunctionType.Copy,
                scale=rec[:],
            )
            o2 = opool.tile([P, DOUT], F32)
            nc.vector.tensor_add(o2[:], o1[:], bxb[:])
            nc.scalar.dma_start(out_d[ti * P : (ti + 1) * P, :], o2[:])

    nc.compile()
    return nc


def host_inputs(x, mask, Wc, Wcat, Wx, bx, b):
    """Per-core input map for batch b (weights replicated, host-prepped)."""
    import ml_dtypes

    wc = np.concatenate([Wx.T, Wc.T], axis=1).astype(ml_dtypes.bfloat16)
    return {
        "x": np.ascontiguousarray(x[b], dtype=np.float32),
        "mask": np.ascontiguousarray(mask[b], dtype=np.int32),
        "wcomb": np.ascontiguousarray(wc),
        "a2": np.ascontiguousarray(
            np.broadcast_to(Wcat[DA:].reshape(1, DA), (P, DA)), dtype=np.float32
        ),
        "bx": np.ascontiguousarray(
            np.broadcast_to(bx.reshape(1, DOUT), (P, DOUT)), dtype=np.float32
        ),
        "ident": np.eye(P, dtype=ml_dtypes.bfloat16),
    }


_cached = {}


def _get_nc():
    if "nc" not in _cached:
        _cached["nc"] = build()
    return _cached["nc"]


def _install_ntff_shim():
    """The agent image's antenv lacks axon_hooks; synthesize it so
    run_bass_kernel_spmd(trace=True) can reach the .so's NTFF profiler."""
    import types

    try:
        import antenv.axon_hooks  # noqa: F401

        return True
    except ImportError:
        pass
    try:
        import antenv
        from trn_agent_boot.trn_boot import _ntff_profile_via_ctypes

        hook = _ntff_profile_via_ctypes("/opt/axon/libaxon_pjrt.so")
        mod = types.ModuleType("antenv.axon_hooks")
        _state = {"hook": hook}
        mod.set_axon_ntff_profile_hook = lambda h: _state.__setitem__("hook", h)
        mod.get_axon_ntff_profile_hook = lambda: _state["hook"]
        sys.modules["antenv.axon_hooks"] = mod
        antenv.axon_hooks = mod
        return hook is not None
    except Exception as e:
        print(f"ntff shim failed: {e}", file=sys.stderr)
        return False


def kernel(x, mask, Wr, Wc, Wcat, Wx, bx, _trace=False, **_unused):
    x = np.asarray(x)
    mask = np.asarray(mask)
    Wc = np.asarray(Wc)
    Wcat = np.asarray(Wcat)
    Wx = np.asarray(Wx)
    bx = np.asarray(bx)
    nc = _get_nc()
    if _trace:
        _trace = _install_ntff_shim()
    in_maps = [host_inputs(x, mask, Wc, Wcat, Wx, bx, b) for b in range(B)]
    res = run_bass_kernel_spmd(nc, in_maps, core_ids=list(range(B)), trace=_trace)
    out = np.stack([res.results[c]["out"] for c in range(B)]).astype(np.float32)
    if _trace:
        kernel.last_results = res
    return out



# revision 2
# speedup vs baseline: 1.1535x; 1.1535x over previous
"""GAT-style attention kernel for Trainium2, data-parallel over batch on 8 cores.

Math (same derivation as baseline): rank-1 score structure makes lr_row cancel
in the softmax, so
    out = (M @ (w * xv0)) / (M @ w) + bx,   w[j] = exp(lr_col[j]), xv0 = x @ Wx.T
(no max-subtraction needed: lr_col is O(1), exp cannot overflow).

v2 design vs baseline: the mask transpose moves OFF the DMA fabric (the xbar
transpose generated ~265B packets that poisoned the shared SDMA engines and
held mask loads to ~145GB/s).  Per strip ti:
  - sync HWDGE loads mask strip pairs [128, 2, N] int32 (2MB per dma)
  - DVE casts one strip [128, N] i32->bf16 (2x perf mode)
  - PE transposes the strip: 16 plain matmuls lhsT=mask chunk, rhs=identity
    -> bf16 PSUM tiles [128, 512] (4 chunks each), evacuated to SBUF by
    DVE/ACT alternating (bf16 keeps DVE in 2x mode)
  - PE accumulates pacc[i, 132] over 16 chunks: lhsT=mT chunk, rhs=U chunk
    (U[:,0:128]=w*xv0, U[:,128]=w), interleaved with the next strip's
    transpose matmuls so the PE never waits on evacuation
  - phase B per strip straight from PSUM: DVE reciprocal of the denom col,
    ACT scale-copy, DVE +bx, SWDGE (gpsimd) store
DMA then carries only the compulsory 18MB/core -> memory roofline ~50us.
"""

import os
import sys

import numpy as np

for _p in ("/opt/trn_rl_repo",):
    if _p not in sys.path and os.path.isdir(_p):
        sys.path.append(_p)

import concourse.bacc as bacc
import concourse.bass as bass
import concourse.tile as tile
from concourse import mybir
from concourse.bass_utils import run_bass_kernel_spmd

B, N, DIN, DOUT, DA = 8, 2048, 128, 128, 2
NEG_SLOPE = 0.2
P = 128
UC = 132  # U chunk width: 128 numerator cols + 1 denom col + 3 pad

F32 = mybir.dt.float32
BF16 = mybir.dt.bfloat16
I32 = mybir.dt.int32


def build(n=N, mask_bufs=3, cast_bufs=6, mt_bufs=4, pair=2, phaseb_skew=1,
          m_skew=2, cast="swdge",
          evac_pat=("vector", "scalar", "vector", "scalar"), tmode="xpose_bf16"):
    """Build the single-core program (all 8 cores run it SPMD).

    cast:  "swdge" — gpsimd cast-during-DMA loads (i32->bf16 inline, no DVE cast)
           "dve"   — sync HWDGE pair loads + DVE tensor_copy cast
    tmode: "xpose_bf16" — transpose-mode matmuls into BF16 PSUM (DVE evacs 2x)
           "mm_f32"     — plain matmuls vs identity into F32 PSUM
    m_skew: M-phase of strip k runs in iteration k+m_skew (pipeline depth)
    """
    nt = n // P
    assert nt % 4 == 0 and nt % pair == 0
    ng = nt // 4  # transpose groups of 4 chunks per strip
    nc = bacc.Bacc(
        "TRN2",
        target_bir_lowering=False,
        debug=False,
        enable_asserts=False,
        num_devices=1,
    )
    xbf_d = nc.dram_tensor("xbf", [P, nt, DIN], BF16, kind="ExternalInput").ap()
    m_d = nc.dram_tensor("mask", [n, n], I32, kind="ExternalInput").ap()
    wxT_d = nc.dram_tensor("wxT", [DIN, DOUT], BF16, kind="ExternalInput").ap()
    wcT_d = nc.dram_tensor("wcT", [DIN, DA], BF16, kind="ExternalInput").ap()
    a2_d = nc.dram_tensor("a2", [P, DA], F32, kind="ExternalInput").ap()
    bx_d = nc.dram_tensor("bx", [P, DOUT], F32, kind="ExternalInput").ap()
    ident_d = nc.dram_tensor("ident", [P, P], BF16, kind="ExternalInput").ap()
    out_d = nc.dram_tensor("out", [n, DOUT], F32, kind="ExternalOutput").ap()

    m_v = m_d.rearrange("(q p) j -> p q j", p=P)  # [P, nt, n] strip view

    from contextlib import ExitStack

    with tile.TileContext(nc) as tc, ExitStack() as ctx:
        consts = ctx.enter_context(tc.tile_pool(name="consts", bufs=1))
        small = ctx.enter_context(tc.tile_pool(name="small", bufs=2))
        if cast == "dve":
            mpool = ctx.enter_context(tc.tile_pool(name="mpool", bufs=mask_bufs))
        cpool = ctx.enter_context(tc.tile_pool(name="cpool", bufs=cast_bufs))
        tpool = ctx.enter_context(tc.tile_pool(name="tpool", bufs=mt_bufs))
        opool = ctx.enter_context(tc.tile_pool(name="opool", bufs=4))
        ps_x = ctx.enter_context(tc.tile_pool(name="ps_x", bufs=3, space="PSUM"))
        ps_acc = ctx.enter_context(tc.tile_pool(name="ps_acc", bufs=3, space="PSUM"))
        ps_pxv = ctx.enter_context(tc.tile_pool(name="ps_pxv", bufs=1, space="PSUM"))

        ev = {"vector": nc.vector.tensor_copy, "scalar": nc.scalar.copy}
        psx_dt = BF16 if tmode == "xpose_bf16" else F32

        def pe_transpose(out_ps, in_sb):
            if tmode == "xpose_bf16":
                nc.tensor.transpose(out_ps, in_sb, identB[:])
            else:
                nc.tensor.matmul(out_ps, in_sb, identB[:], start=True, stop=True)

        # ---- constants on scalar queue; x on whichever HWDGE queue is idle ----
        xq = nc.sync if cast == "swdge" else nc.scalar
        xbf = consts.tile([P, nt, DIN], BF16)
        nxc = max(1, nt // 4)
        for c in range(nxc):
            lo = c * (nt // nxc)
            hi = (c + 1) * (nt // nxc)
            xq.dma_start(xbf[:, lo:hi], xbf_d[:, lo:hi])
        identB = consts.tile([P, P], BF16)
        nc.scalar.dma_start(identB[:], ident_d)
        wxT = consts.tile([DIN, DOUT], BF16)
        nc.scalar.dma_start(wxT[:], wxT_d)
        wcT = consts.tile([DIN, DA], BF16)
        nc.scalar.dma_start(wcT[:], wcT_d)
        a2b = consts.tile([P, DA], F32)
        nc.scalar.dma_start(a2b[:], a2_d)
        bxb = consts.tile([P, DOUT], F32)
        nc.scalar.dma_start(bxb[:], bx_d)

        # ---- x -> xT via plain PE matmuls against identity, packed 4/psum tile ----
        xT = consts.tile([P, n], BF16)
        for g in range(ng):
            psx = ps_x.tile([P, 4 * P], psx_dt, tag="psx")
            for q in range(4):
                t = 4 * g + q
                pe_transpose(psx[:, q * P : (q + 1) * P], xbf[:, t])
            ev[evac_pat[g % len(evac_pat)]](
                xT[:, 4 * g * P : (4 * g + 4) * P], psx[:]
            )

        # ---- col projection (N=2) first so the w chain runs early ----
        pcol = ps_pxv.tile([P, nt * DA], F32, tag="pcol")
        for t in range(nt):
            nc.tensor.matmul(
                pcol[:, t * DA : (t + 1) * DA],
                xT[:, t * P : (t + 1) * P],
                wcT[:],
                start=True,
                stop=True,
            )
        colv = small.tile([P, nt, DA], F32)
        nc.vector.tensor_copy(colv[:], pcol[:].rearrange("p (t a) -> p t a", a=DA))
        c02 = small.tile([P, nt, DA], F32)
        nc.vector.tensor_scalar_mul(c02[:], colv[:], NEG_SLOPE)
        clr = small.tile([P, nt, DA], F32)
        nc.vector.tensor_max(clr[:], colv[:], c02[:])
        lr0 = small.tile([P, nt], F32)
        nc.vector.tensor_scalar(
            lr0[:], clr[:, :, 0], a2b[:, 0:1], None, mybir.AluOpType.mult
        )
        lr1 = small.tile([P, nt], F32)
        nc.vector.tensor_scalar(
            lr1[:], clr[:, :, 1], a2b[:, 1:2], None, mybir.AluOpType.mult
        )
        lrc = small.tile([P, nt], F32)
        nc.vector.tensor_add(lrc[:], lr0[:], lr1[:])
        w_all = consts.tile([P, nt], F32)
        nc.scalar.activation(w_all[:], lrc[:], mybir.ActivationFunctionType.Exp)

        # ---- U chunks [P, nt, UC] bf16: U[:,t,0:128]=w*xv0, U[:,t,128]=w ----
        U = consts.tile([P, nt, UC], BF16)
        nc.vector.memset(U[:], 0)
        for g in range(ng):
            psv = ps_pxv.tile([P, 4 * P], F32, tag="psv")
            for q in range(4):
                t = 4 * g + q
                nc.tensor.matmul(
                    psv[:, q * P : (q + 1) * P],
                    xT[:, t * P : (t + 1) * P],
                    wxT[:],
                    start=True,
                    stop=True,
                )
            for q in range(4):
                t = 4 * g + q
                if q % 2 == 0:
                    nc.scalar.activation(
                        U[:, t, 0:DOUT],
                        psv[:, q * P : (q + 1) * P],
                        mybir.ActivationFunctionType.Copy,
                        scale=w_all[:, t : t + 1],
                    )
                else:
                    nc.vector.tensor_scalar(
                        U[:, t, 0:DOUT],
                        psv[:, q * P : (q + 1) * P],
                        w_all[:, t : t + 1],
                        None,
                        mybir.AluOpType.mult,
                    )
        nc.vector.tensor_copy(U[:, :, DOUT], w_all[:])

        # ---- main loop: software-pipelined; iter k runs T(k) + M(k-m_skew) ----
        mis = {}
        mbfs = {}
        mTs = {}
        paccs = {}
        store_q = nc.sync if cast == "swdge" else nc.gpsimd

        def phase_b(ti):
            pacc = paccs.pop(ti)
            rec = small.tile([P, 1], F32, tag="rec")
            nc.vector.reciprocal(rec[:], pacc[:, DOUT : DOUT + 1])
            o1 = opool.tile([P, DOUT], F32, tag="o1")
            nc.scalar.activation(
                o1[:], pacc[:, 0:DOUT], mybir.ActivationFunctionType.Copy,
                scale=rec[:],
            )
            o2 = opool.tile([P, DOUT], F32, tag="o2")
            nc.vector.tensor_add(o2[:], o1[:], bxb[:])
            store_q.dma_start(out_d[ti * P : (ti + 1) * P, :], o2[:])

        for k in range(nt + m_skew):
            if k < nt:
                if cast == "swdge":
                    mbf = cpool.tile([P, n], BF16, tag="mbf")
                    nc.gpsimd.dma_start(mbf[:], m_v[:, k])
                    mbfs[k] = mbf
                else:
                    if k % pair == 0:
                        mi = mpool.tile([P, pair, n], I32, tag="mi")
                        nc.sync.dma_start(mi[:], m_v[:, k : k + pair])
                        for a in range(pair):
                            mis[k + a] = (mi, a)
                    mi, a = mis.pop(k)
                    mbf = cpool.tile([P, n], BF16, tag="mbf")
                    nc.vector.tensor_copy(mbf[:], mi[:, a])
                    mbfs[k] = mbf
                mTs[k] = tpool.tile([P, n], BF16, name="mT", tag="mT")
            if k >= m_skew + phaseb_skew:
                phase_b(k - m_skew - phaseb_skew)
            if k >= m_skew:
                paccs[k - m_skew] = ps_acc.tile([P, UC], F32, name="pacc", tag="pacc")
            for g in range(ng):
                if k < nt:
                    mbf = mbfs[k]
                    psx = ps_x.tile([P, 4 * P], psx_dt, tag="psx")
                    for q in range(4):
                        t = 4 * g + q
                        pe_transpose(
                            psx[:, q * P : (q + 1) * P],
                            mbf[:, t * P : (t + 1) * P],
                        )
                if k >= m_skew:
                    mT = mTs[k - m_skew]
                    for q in range(4):
                        tj = 4 * g + q
                        nc.tensor.matmul(
                            paccs[k - m_skew][:],
                            mT[:, tj * P : (tj + 1) * P],
                            U[:, tj],
                            start=(tj == 0),
                            stop=(tj == nt - 1),
                        )
                if k < nt:
                    ev[evac_pat[g % len(evac_pat)]](
                        mTs[k][:, 4 * g * P : (4 * g + 4) * P], psx[:]
                    )
            if k < nt:
                mbfs.pop(k)
            if k >= m_skew:
                mTs.pop(k - m_skew)
        for ti in range(nt - phaseb_skew, nt):
            phase_b(ti)

    nc.compile()
    return nc


def host_inputs(x, mask, Wc, Wcat, Wx, bx, b):
    """Per-core input map for batch b (weights replicated, host-prepped)."""
    import ml_dtypes

    nt = N // P
    xb = np.asarray(x[b], dtype=np.float32)
    xprep = xb.reshape(nt, P, DIN).transpose(1, 0, 2)  # [P, nt, DIN]
    return {
        "xbf": np.ascontiguousarray(xprep, dtype=ml_dtypes.bfloat16),
        "mask": np.ascontiguousarray(mask[b], dtype=np.int32),
        "wxT": np.ascontiguousarray(Wx.T, dtype=ml_dtypes.bfloat16),
        "wcT": np.ascontiguousarray(Wc.T, dtype=ml_dtypes.bfloat16),
        "a2": np.ascontiguousarray(
            np.broadcast_to(Wcat[DA:].reshape(1, DA), (P, DA)), dtype=np.float32
        ),
        "bx": np.ascontiguousarray(
            np.broadcast_to(bx.reshape(1, DOUT), (P, DOUT)), dtype=np.float32
        ),
        "ident": np.eye(P, dtype=ml_dtypes.bfloat16),
    }


_cached = {}


def _get_nc():
    if "nc" not in _cached:
        _cached["nc"] = build()
    return _cached["nc"]


def _install_ntff_shim():
    """The agent image's antenv lacks axon_hooks; synthesize it so
    run_bass_kernel_spmd(trace=True) can reach the .so's NTFF profiler."""
    import types

    try:
        import antenv.axon_hooks  # noqa: F401

        return True
    except ImportError:
        pass
    try:
        import antenv
        from trn_agent_boot.trn_boot import _ntff_profile_via_ctypes

        hook = _ntff_profile_via_ctypes("/opt/axon/libaxon_pjrt.so")
        mod = types.ModuleType("antenv.axon_hooks")
        _state = {"hook": hook}
        mod.set_axon_ntff_profile_hook = lambda h: _state.__setitem__("hook", h)
        mod.get_axon_ntff_profile_hook = lambda: _state["hook"]
        sys.modules["antenv.axon_hooks"] = mod
        antenv.axon_hooks = mod
        return hook is not None
    except Exception as e:
        print(f"ntff shim failed: {e}", file=sys.stderr)
        return False


def kernel(x, mask, Wr, Wc, Wcat, Wx, bx, _trace=False, **_unused):
    x = np.asarray(x)
    mask = np.asarray(mask)
    Wc = np.asarray(Wc)
    Wcat = np.asarray(Wcat)
    Wx = np.asarray(Wx)
    bx = np.asarray(bx)
    nc = _get_nc()
    if _trace:
        _trace = _install_ntff_shim()
    in_maps = [host_inputs(x, mask, Wc, Wcat, Wx, bx, b) for b in range(B)]
    res = run_bass_kernel_spmd(nc, in_maps, core_ids=list(range(B)), trace=_trace)
    out = np.stack([res.results[c]["out"] for c in range(B)]).astype(np.float32)
    if _trace:
        kernel.last_results = res
    return out
